# revision 8
# baseline (speedup 1.0000x reference)
"""Trainium2 Bass kernel for nn_ChessGraphPooling (segment_reduce).

Data-parallel over whole graphs: 4096 boards x 64 nodes sharded across 8
NeuronCores (512 graphs / 32768 nodes per core); small weights replicated.

v2 design (vs. the streaming fp32r baseline):
  - x is shipped from the host in TWO layouts: node-layout bf16 [nodes, C]
    (pooling matmuls) and transposed fp8-e4m3 DoubleRow pairs
    [128, 2, nodes] (all per-node linears), eliminating all on-chip x
    transposes/copies and halving HBM traffic.
  - the three per-node matmuls (attention scorer hidden, piece/empty hidden,
    strategic projection) and the tiny scorer second layers run as fp8
    DoubleRow matmuls (2 fp8 contraction elements per PE cell).  Weights are
    scaled x16 on the host so they sit in fp8's normal range; the x256 score
    scale is removed inside the softmax exp, and the x16 strategic scale is
    absorbed exactly by the LayerNorm (eps adjusted by 256).
  - pooling uses x/sf as the matmul *stationary* operand so pooled features
    land directly in T-layout PSUM; they are staged to SBUF by DMA, not by
    engine copies.
  - scores are staged PSUM->SBUF by DMA as well; segment softmax is batched
    [80 rows, 8 graphs, 64 nodes] per megatile in bf16.
  - the strategic LN uses bn_stats then a single fused
    relu(zs*rho - mu*rho) activation per chunk on the scalar engine.
  - biases/LN affines that are all-zero/identity in this model are checked on
    the host; nonzero values compile a general (slower) variant.
"""

import os
import sys

sys.path.insert(0, "/opt/trn_rl_repo")

from contextlib import ExitStack

import numpy as np
import ml_dtypes

import concourse.bass as bass
import concourse.bacc as bacc
import concourse.tile as tile
import concourse.mybir as mybir
from concourse.bass_utils import run_bass_kernel_spmd
from concourse.masks import make_identity

F32 = mybir.dt.float32
F32R = mybir.dt.float32r
BF16 = mybir.dt.bfloat16
F8 = mybir.dt.float8e4
I32 = mybir.dt.int32
AF = mybir.ActivationFunctionType
OP = mybir.AluOpType
AX = mybir.AxisListType
DR = mybir.MatmulPerfMode.DoubleRow

C = 256
H = 8
NODES = 64
NEG = 0.2
N_CORES = 8
ST = 512          # nodes per supertile
CHUNKS = 4        # 128-node chunks per supertile
MEGA = 8          # supertiles per megatile (80 score rows)
FULL_N_MEGA = 8   # megatiles per core at full size
WSCALE = 16.0     # host-side fp8 weight scale

# engine split for the 6 scorer-hidden activations (a=Act, d=DVE)
HL_ENG = os.environ.get("K_HL_ENG", "aadaad")


def _r(ap):
    return ap.bitcast(F32R)


def build_nc(n_mega=FULL_N_MEGA, flags=frozenset()):
    nodes_pc = n_mega * MEGA * ST
    graphs_pc = nodes_pc // NODES
    assert graphs_pc % 128 == 0, "post stage needs graphs_pc multiple of 128"

    nc = bacc.Bacc("TRN2", num_devices=N_CORES)

    dt = {}

    def din(name, shape, dtype=F32):
        dt[name] = nc.dram_tensor(name, shape, dtype, kind="ExternalInput")

    din("xT8", [128, 2, nodes_pc], F8)
    din("xbf", [nodes_pc, C], BF16)
    din("nt", [nodes_pc], I32)
    din("w1dr", [6, 128, 2, 128], F8)
    din("w2dr", [3, 128, 2, 32], F8)
    din("spw8", [128, 2, 256], F8)
    din("b1c", [128, 6])
    din("b2c", [32, 1])
    din("spb16", [1, 256], F32R)
    din("spg", [1, 256])
    din("spbt", [1, 256])
    din("sw", [64, 1])
    # post-stage weights (identical to baseline layout)
    din("cw", [16, 128, 256], F32R)
    din("cb", [1, 256], F32R)
    din("cg", [1, 256])
    din("cbt", [1, 256])
    din("hw", [4, 128, 256], F32R)
    din("hb", [1, 256], F32R)
    din("hg", [1, 256])
    din("hbt", [1, 256])
    din("p1w", [6, 128, 512], F32R)
    din("p1b", [1, 512], F32R)
    din("p1g", [1, 512])
    din("p1bt", [1, 512])
    din("p2w", [4, 128, 256], F32R)
    din("p2b", [1, 256], F32R)
    out_d = nc.dram_tensor("out", [graphs_pc, C], F32, kind="ExternalOutput")

    with tile.TileContext(nc) as tc:
        _build_body(nc, tc, n_mega, graphs_pc, dt, out_d, flags)
    nc.compile()
    return nc


def _bcast(nc, dst, src_d):
    nc.gpsimd.dma_start(
        out=dst, in_=src_d.ap().partition_broadcast(dst.shape[0])
    )


def _build_body(nc, tc, n_mega, graphs_pc, dt, out_d, flags):
    gchunks = graphs_pc // 128

    with ExitStack() as top:
        consts = top.enter_context(tc.tile_pool(name="consts", bufs=1))
        persist = top.enter_context(tc.tile_pool(name="persist", bufs=1))

        # ---- constants ----
        w1t = [consts.tile([128, 2, 128], F8, tag=f"w1t{m}", name=f"w1t{m}")
               for m in range(6)]
        for m in range(6):
            nc.sync.dma_start(out=w1t[m], in_=dt["w1dr"].ap()[m])
        w2t = [consts.tile([128, 2, 32], F8, tag=f"w2t{p}", name=f"w2t{p}")
               for p in range(3)]
        for p in range(3):
            nc.sync.dma_start(out=w2t[p], in_=dt["w2dr"].ap()[p])
        spwt = consts.tile([128, 2, 256], F8, tag="spwt")
        nc.sync.dma_start(out=spwt, in_=dt["spw8"].ap())
        b1c = consts.tile([128, 6], F32, tag="b1c")
        nc.sync.dma_start(out=b1c, in_=dt["b1c"].ap())
        b2c = consts.tile([32, 1], F32, tag="b2c")
        nc.sync.dma_start(out=b2c, in_=dt["b2c"].ap())
        spbr = consts.tile([1, 256], F32R, tag="spbr")
        nc.sync.dma_start(out=spbr, in_=dt["spb16"].ap())
        onesf = consts.tile([1, 128], F32, tag="onesf")
        nc.vector.memset(onesf, 1.0)
        ones1 = consts.tile([1, 128], F32R, tag="ones1")
        nc.vector.tensor_copy(out=ones1, in_=onesf)

        gB = consts.tile([128, 256], F32, tag="gB")
        _bcast(nc, gB, dt["spg"])
        btB = consts.tile([128, 256], F32, tag="btB")
        _bcast(nc, btB, dt["spbt"])

        sa = consts.tile([128, 1], F32, tag="sa")
        nc.sync.dma_start(out=sa[0:64, :], in_=dt["sw"].ap())
        nc.sync.dma_start(out=sa[64:128, :], in_=dt["sw"].ap())
        sa2 = consts.tile([128, 1], F32, tag="sa2")
        nc.vector.tensor_tensor(out=sa2, in0=sa, in1=sa, op=OP.mult)

        poolf = consts.tile([128, 2], F32, tag="poolf")
        nc.gpsimd.memset(poolf, 0.0)
        nc.gpsimd.memset(poolf[0:64, 0:1], 1.0 / NODES)
        nc.gpsimd.memset(poolf[64:128, 1:2], 1.0 / NODES)
        poolcol = consts.tile([128, 2], BF16, tag="poolcol")
        nc.vector.tensor_copy(out=poolcol, in_=poolf)

        identf = consts.tile([128, 128], F32, tag="identf")
        make_identity(nc, identf)
        identB = consts.tile([128, 128], BF16, tag="identB")
        nc.vector.tensor_copy(out=identB, in_=identf)

        maskS = consts.tile([80, 512], F32, tag="maskS")
        nc.vector.memset(maskS, 1.0)

        # ---- persistent staging (alive into the post stage) ----
        staged = persist.tile([128, 2, graphs_pc * 11], F32, tag="staged")

        with ExitStack() as main:
            xtp = main.enter_context(tc.tile_pool(name="xtp", bufs=2))
            xbp = main.enter_context(tc.tile_pool(name="xbp", bufs=2))
            hlp = main.enter_context(tc.tile_pool(name="hlp", bufs=9))
            sfp = main.enter_context(tc.tile_pool(name="sfp", bufs=9))
            megap = main.enter_context(tc.tile_pool(name="megap", bufs=2))
            smp = main.enter_context(tc.tile_pool(name="smp", bufs=3))
            wcp = main.enter_context(tc.tile_pool(name="wcp", bufs=3))
            stp = main.enter_context(tc.tile_pool(name="stp", bufs=3))

            ps_ph = main.enter_context(
                tc.tile_pool(name="ps_ph", bufs=2, space="PSUM"))
            ps_sc = main.enter_context(
                tc.tile_pool(name="ps_sc", bufs=2, space="PSUM"))
            ps_pz = main.enter_context(
                tc.tile_pool(name="ps_pz", bufs=2, space="PSUM"))
            ps_pt = main.enter_context(
                tc.tile_pool(name="ps_pt", bufs=1, space="PSUM"))
            ps_tp = main.enter_context(
                tc.tile_pool(name="ps_tp", bufs=1, space="PSUM"))

            for mega in range(n_mega):
                _mega_body(
                    nc, tc, mega, dt, staged, flags,
                    w1t, w2t, spwt, b1c, b2c, spbr, gB, btB, sa, sa2,
                    poolcol, identB, maskS, ones1,
                    xtp, xbp, hlp, sfp, megap, smp, wcp, stp,
                    ps_ph, ps_sc, ps_pz, ps_pt, ps_tp,
                )

        # ---- post stage ----
        with ExitStack() as post:
            posw = post.enter_context(tc.tile_pool(name="posw", bufs=1))
            pos = post.enter_context(tc.tile_pool(name="pos", bufs=4))
            posT = post.enter_context(tc.tile_pool(name="posT", bufs=1))
            ps_po = post.enter_context(
                tc.tile_pool(name="ps_po", bufs=2, space="PSUM"))
            ps_pz2 = post.enter_context(
                tc.tile_pool(name="ps_pz2", bufs=2, space="PSUM"))
            ps_pt2 = post.enter_context(
                tc.tile_pool(name="ps_pt2", bufs=2, space="PSUM"))
            _post_body(
                nc, tc, graphs_pc, gchunks, dt, staged,
                ones1, identf, posw, pos, posT, ps_po, ps_pz2, ps_pt2, out_d,
            )


def _mega_body(
    nc, tc, mega, dt, staged, flags,
    w1t, w2t, spwt, b1c, b2c, spbr, gB, btB, sa, sa2,
    poolcol, identB, maskS, ones1,
    xtp, xbp, hlp, sfp, megap, smp, wcp, stp,
    ps_ph, ps_sc, ps_pz, ps_pt, ps_tp,
):
    mst = mega * MEGA          # first supertile of this mega
    mn0 = mst * ST             # first node

    # ---- input DMAs ----
    xT8m = xtp.tile([128, 2, MEGA * ST], F8, tag="xT8m")
    nc.sync.dma_start(
        out=xT8m, in_=dt["xT8"].ap()[:, :, mn0:mn0 + MEGA * ST]
    )
    xbfm = xbp.tile([128, MEGA * CHUNKS, 256], BF16, tag="xbfm")
    nc.sync.dma_start(
        out=xbfm,
        in_=dt["xbf"].ap()[mn0:mn0 + MEGA * ST, :]
        .rearrange("(s p) m -> p s m", p=128),
    )

    # masks: node_types -> maskS rows 10*j+8 (m) / 10*j+9 (1-m)
    ntm = megap.tile([8, 512], I32, tag="ntm")
    nc.sync.dma_start(
        out=ntm,
        in_=dt["nt"].ap()[mn0:mn0 + MEGA * ST].rearrange("(s n) -> s n", s=8),
    )
    m8 = megap.tile([8, 512], F32, tag="m8")
    nc.vector.tensor_copy(out=m8, in_=ntm)
    n8 = megap.tile([8, 512], F32, tag="n8")
    nc.vector.tensor_scalar(
        out=n8, in0=m8, scalar1=-1.0, scalar2=1.0, op0=OP.mult, op1=OP.add
    )
    nc.gpsimd.dma_start(out=maskS[8:80:10, :], in_=m8)
    nc.gpsimd.dma_start(out=maskS[9:80:10, :], in_=n8)

    # ---- phase 1: scorer hidden layers (fp8 DoubleRow), M-chunk outer ----
    hl8 = [hlp.tile([128, 3, 2, 512], F8, tag="hl8", name=f"hl8_{s}")
           for s in range(MEGA)]
    for m in range(6):
        pair, half = divmod(m, 2) if m < 4 else (2, m - 4)
        for s in range(MEGA):
            ph = ps_ph.tile([128, 512], F32, tag="ph")
            for h in range(2):
                nc.tensor.matmul(
                    ph[:, h * 256:(h + 1) * 256], w1t[m],
                    xT8m[:, :, s * ST + h * 256:s * ST + (h + 1) * 256],
                    start=True, stop=True, perf_mode=DR,
                )
            dst = hl8[s][:, pair, half, :]
            eng = HL_ENG[m] if "b1" not in flags else "a"
            if eng == "a":
                nc.scalar.activation(
                    out=dst, in_=ph, func=AF.Prelu,
                    bias=(b1c[:, m:m + 1] if "b1" in flags else 0.0),
                    scale=1.0, alpha=NEG,
                )
            else:
                nc.vector.scalar_tensor_tensor(
                    out=dst, in0=ph, scalar=NEG, in1=ph,
                    op0=OP.mult, op1=OP.max,
                )

    # ---- phase 2: scorer second layers -> scores (fp8 DoubleRow) ----
    scstack = megap.tile([80, 512], F32, tag="scstack")
    for s in range(MEGA):
        sct = ps_sc.tile([32, 512], F32, tag="sct")
        for p in range(3):
            for h in range(2):
                nc.tensor.matmul(
                    sct[:, h * 256:(h + 1) * 256], w2t[p],
                    hl8[s][:, p, :, h * 256:(h + 1) * 256],
                    start=(p == 0), stop=(p == 2), perf_mode=DR,
                )
        stmp = smp.tile([10, 512], F32, tag="stmp")
        isl = sct[0:10, :]
        if "b2" in flags:
            nc.scalar.activation(
                out=stmp, in_=isl, func=AF.Identity,
                bias=b2c[0:10, :], scale=1.0,
            )
        elif s % 2 == 0:
            nc.vector.tensor_copy(out=stmp, in_=isl)
        else:
            nc.scalar.copy(out=stmp, in_=isl)
        nc.sync.dma_start(out=scstack[s * 10:(s + 1) * 10, :], in_=stmp)

    # ---- phase 3: batched segment softmax (scores are 256x scaled) ----
    nc.gpsimd.tensor_tensor(
        out=scstack, in0=scstack, in1=maskS, op=OP.mult
    )
    sc3 = scstack.rearrange("p (g n) -> p g n", n=NODES)
    mx = megap.tile([80, 8], F32, tag="mx")
    nc.vector.tensor_reduce(out=mx, in_=sc3, axis=AX.X, op=OP.max)
    wsub = megap.tile([80, 512], F32, tag="wsub")
    nc.vector.tensor_tensor(
        out=wsub.rearrange("p (g n) -> p g n", n=NODES), in0=sc3,
        in1=mx.unsqueeze(2).broadcast_to([80, 8, NODES]),
        op=OP.subtract,
    )
    wT = megap.tile([80, 512], BF16, tag="wT")
    nc.scalar.activation(out=wT, in_=wsub, func=AF.Exp, scale=1.0 / 256.0)
    wT3 = wT.rearrange("p (g n) -> p g n", n=NODES)
    dsum = megap.tile([80, 8], F32, tag="dsum")
    nc.vector.tensor_reduce(out=dsum, in_=wT3, axis=AX.X, op=OP.add)
    nc.vector.tensor_scalar(
        out=dsum, in0=dsum, scalar1=1e-16, scalar2=None, op0=OP.add,
    )
    nc.vector.reciprocal(out=dsum, in_=dsum)
    dsb = megap.tile([80, 8], BF16, tag="dsb")
    nc.vector.tensor_copy(out=dsb, in_=dsum)
    nc.vector.tensor_tensor(
        out=wT3, in0=wT3,
        in1=dsb.unsqueeze(2).broadcast_to([80, 8, NODES]),
        op=OP.mult,
    )

    # transpose the weight stack: wtt[:, c, r] = wT[r, c*128+:]
    wtt = megap.tile([128, 4, 80], BF16, tag="wtt")
    for c in range(CHUNKS):
        tp = ps_tp.tile([128, 80], BF16, tag="tp")
        nc.tensor.transpose(
            tp, wT[:, c * 128:(c + 1) * 128], identB[0:80, 0:80]
        )
        nc.vector.tensor_copy(out=wtt[:, c, :], in_=tp)

    # ---- phase 4: strategic branch + pooling, per supertile ----
    mvs = megap.tile([128, MEGA, 4, 2], F32, tag="mvs")
    for s in range(MEGA):
        sg = mst + s
        # 4a: strat matmuls + LN stats
        pzs = []
        for cp in range(2):
            pz = ps_pz.tile([128, 2, 256], F32, tag="pz")
            pzs.append(pz)
            for cc in range(2):
                c = cp * 2 + cc
                psl = pz[:, cc, :]
                nc.tensor.matmul(
                    psl,
                    xT8m[:, :, s * ST + c * 128:s * ST + (c + 1) * 128],
                    spwt, start=True, stop=("spb" not in flags),
                    perf_mode=DR,
                )
                if "spb" in flags:
                    nc.tensor.matmul(
                        psl, _r(ones1), _r(spbr), start=False, stop=True,
                        skip_group_check=True,
                    )
                st6 = smp.tile([128, 6], F32, tag="st6")
                nc.vector.bn_stats(out=st6, in_=psl)
                nc.vector.bn_aggr(out=mvs[:, s, c, :], in_=st6)
        # rho chain for this supertile: rho = sa*rsqrt(sa^2*var' + 256*eps)
        rho = smp.tile([128, 4], F32, tag="rho")
        nc.vector.tensor_scalar(
            out=rho, in0=mvs[:, s, :, 1], scalar1=sa2, scalar2=256e-5,
            op0=OP.mult, op1=OP.add,
        )
        nc.scalar.activation(out=rho, in_=rho, func=AF.Sqrt)
        nc.vector.reciprocal(out=rho, in_=rho)
        nc.vector.tensor_scalar(
            out=rho, in0=rho, scalar1=sa, scalar2=None, op0=OP.mult
        )
        nmr = smp.tile([128, 4], F32, tag="nmr")
        nc.vector.tensor_tensor(out=nmr, in0=mvs[:, s, :, 0], in1=rho,
                                op=OP.mult)
        nc.vector.tensor_scalar(
            out=nmr, in0=nmr, scalar1=-1.0, scalar2=None, op0=OP.mult
        )
        # 4b: sf = relu((zs - mu) * rho) (+ general gamma/beta path)
        sf = sfp.tile([128, 4, 256], BF16, tag="sf", name=f"sf_{s}")
        for c in range(CHUNKS):
            psl = pzs[c // 2][:, c % 2, :]
            if "spg" not in flags:
                nc.scalar.activation(
                    out=sf[:, c, :], in_=psl, func=AF.Relu,
                    bias=nmr[:, c:c + 1], scale=rho[:, c:c + 1],
                )
            else:
                t1 = smp.tile([128, 256], F32, tag="t1")
                nc.vector.tensor_scalar(
                    out=t1, in0=psl, scalar1=mvs[:, s, c, 0:1],
                    scalar2=None, op0=OP.subtract,
                )
                nc.vector.tensor_tensor(out=t1, in0=t1, in1=gB, op=OP.mult)
                nc.vector.scalar_tensor_tensor(
                    out=t1, in0=t1, scalar=rho[:, c:c + 1], in1=btB,
                    op0=OP.mult, op1=OP.add,
                )
                nc.scalar.activation(out=sf[:, c, :], in_=t1, func=AF.Relu)

        # pooling weights for this supertile (block-diag over graph pairs)
        wcols = wcp.tile([128, 4, 20], BF16, tag="wcols")
        nc.gpsimd.memset(wcols, 0.0)
        nc.gpsimd.tensor_copy(
            out=wcols[0:64, :, 0:10], in_=wtt[0:64, :, s * 10:(s + 1) * 10]
        )
        nc.gpsimd.tensor_copy(
            out=wcols[64:128, :, 10:20], in_=wtt[64:128, :, s * 10:(s + 1) * 10]
        )

        # pooled features in T-layout, graph-major: pT[:, h, (c*2+gg)*11 + j]
        # (j=0..9 pooled-x heads, j=10 sf mean)
        pT = ps_pt.tile([128, 2, 88], F32, tag="pT")
        pTg = pT.rearrange("p k (g t) -> p k g t", t=11)
        for c in range(CHUNKS):
            for h in range(2):
                nc.tensor.matmul(
                    pTg[:, h, c * 2:(c + 1) * 2, 0:10],
                    xbfm[:, s * 4 + c, h * 128:(h + 1) * 128],
                    wcols[:, c, :], start=True, stop=True,
                )
                nc.tensor.matmul(
                    pTg[:, h, c * 2:(c + 1) * 2, 10:11],
                    sf[:, c, h * 128:(h + 1) * 128],
                    poolcol, start=True, stop=True,
                )
        # stage to SBUF (graph g = sg*8 + c*2 + gg at columns g*11 + j)
        osl = staged[:, :, sg * 88:(sg + 1) * 88]
        if s % 2 == 0:
            nc.scalar.copy(out=osl, in_=pT)
        else:
            nc.vector.tensor_copy(out=osl, in_=pT)


def _post_body(
    nc, tc, graphs_pc, gchunks, dt, staged,
    ones1, ident, posw, pos, posT, ps_po, ps_pz, ps_pt, out_d,
):
    cw = posw.tile([128, 16, 256], F32R, tag="cw")
    nc.sync.dma_start(out=cw, in_=dt["cw"].ap().rearrange("k p c -> p k c"))
    hwt = posw.tile([128, 4, 256], F32R, tag="hwt")
    nc.sync.dma_start(out=hwt, in_=dt["hw"].ap().rearrange("k p c -> p k c"))
    p1w = posw.tile([128, 6, 512], F32R, tag="p1w")
    nc.sync.dma_start(out=p1w, in_=dt["p1w"].ap().rearrange("k p c -> p k c"))
    p2w = posw.tile([128, 4, 256], F32R, tag="p2w")
    nc.sync.dma_start(out=p2w, in_=dt["p2w"].ap().rearrange("k p c -> p k c"))
    cbR = posw.tile([1, 256], F32R, tag="cbR")
    nc.sync.dma_start(out=cbR, in_=dt["cb"].ap())
    hbR = posw.tile([1, 256], F32R, tag="hbR")
    nc.sync.dma_start(out=hbR, in_=dt["hb"].ap())
    p1bR = posw.tile([1, 512], F32R, tag="p1bR")
    nc.sync.dma_start(out=p1bR, in_=dt["p1b"].ap())
    p2bR = posw.tile([1, 256], F32R, tag="p2bR")
    nc.sync.dma_start(out=p2bR, in_=dt["p2b"].ap())
    cgB = posw.tile([128, 256], F32, tag="cgB")
    _bcast(nc, cgB, dt["cg"])
    cbtB = posw.tile([128, 256], F32, tag="cbtB")
    _bcast(nc, cbtB, dt["cbt"])
    hgB = posw.tile([128, 256], F32, tag="hgB")
    _bcast(nc, hgB, dt["hg"])
    hbtB = posw.tile([128, 256], F32, tag="hbtB")
    _bcast(nc, hbtB, dt["hbt"])
    p1gB = posw.tile([128, 512], F32, tag="p1gB")
    _bcast(nc, p1gB, dt["p1g"])
    p1btB = posw.tile([128, 512], F32, tag="p1btB")
    _bcast(nc, p1btB, dt["p1bt"])

    sx3 = staged.rearrange("p k (g t) -> p k g t", t=11)

    catT = [posT.tile([128, graphs_pc], F32R, tag=f"catT{i}", name=f"catT{i}")
            for i in range(4)]
    zT = [posT.tile([128, graphs_pc], F32R, tag=f"zT{i}", name=f"zT{i}")
          for i in range(4)]
    pmv = posT.tile([128, 2 * gchunks, 2], F32, tag="pmv")

    # comb + hier matmuls, LN stats
    cps = []
    for gc in range(gchunks):
        gsl = slice(gc * 128, (gc + 1) * 128)
        cpp = ps_po.tile([128, 256], F32, tag="cpp")
        for h in range(H):
            for k in range(2):
                nc.tensor.matmul(
                    cpp, _r(sx3[:, k, gsl, h]), _r(cw[:, h * 2 + k, :]),
                    start=(h == 0 and k == 0), stop=False,
                )
        nc.tensor.matmul(cpp, _r(ones1), _r(cbR), start=False, stop=True)
        hpp = ps_po.tile([128, 256], F32, tag="cpp")
        for k in range(2):
            nc.tensor.matmul(
                hpp, _r(sx3[:, k, gsl, 8]), _r(hwt[:, k, :]),
                start=(k == 0), stop=False,
            )
            nc.tensor.matmul(
                hpp, _r(sx3[:, k, gsl, 9]), _r(hwt[:, 2 + k, :]),
                start=False, stop=(k == 1),
            )
        nc.tensor.matmul(hpp, _r(ones1), _r(hbR), start=False, stop=True)
        csb = posT.tile([128, 256], F32, tag=f"csb{gc}", name=f"csb{gc}")
        nc.scalar.copy(out=csb, in_=cpp)
        hsb = posT.tile([128, 256], F32, tag=f"hsb{gc}", name=f"hsb{gc}")
        nc.scalar.copy(out=hsb, in_=hpp)
        for i, ppx in enumerate((csb, hsb)):
            st6 = pos.tile([128, 6], F32, tag="pst6")
            nc.vector.bn_stats(out=st6, in_=ppx)
            nc.vector.bn_aggr(out=pmv[:, gc * 2 + i, :], in_=st6)
        cps.append((csb, hsb))

    prr = posT.tile([128, 2 * gchunks], F32, tag="prr")
    nc.vector.tensor_scalar(
        out=prr, in0=pmv[:, :, 1], scalar1=1.0, scalar2=1e-5,
        op0=OP.mult, op1=OP.add,
    )
    nc.scalar.activation(out=prr, in_=prr, func=AF.Sqrt)
    nc.vector.reciprocal(out=prr, in_=prr)

    for gc in range(gchunks):
        gsl = slice(gc * 128, (gc + 1) * 128)
        cpp, hpp = cps[gc]
        for i, (ppx, ggB, bbB) in enumerate(
            ((cpp, cgB, cbtB), (hpp, hgB, hbtB))
        ):
            tg = pos.tile([128, 256], F32, tag="ptg")
            nc.vector.scalar_tensor_tensor(
                out=tg, in0=ppx, scalar=pmv[:, gc * 2 + i, 0:1],
                in1=ggB, op0=OP.subtract, op1=OP.mult,
            )
            nc.vector.scalar_tensor_tensor(
                out=tg, in0=tg, scalar=prr[:, gc * 2 + i:gc * 2 + i + 1],
                in1=bbB, op0=OP.mult, op1=OP.add,
            )
            rg = pos.tile([128, 256], F32, tag="prg")
            nc.scalar.activation(out=rg, in_=tg, func=AF.Relu)
            for cc in range(2):
                ptp = ps_pt.tile([128, 128], F32, tag="pptp")
                nc.tensor.transpose(ptp, rg[:, cc * 128:(cc + 1) * 128], ident)
                nc.vector.tensor_copy(out=catT[i * 2 + cc][:, gsl], in_=ptp)

    catT_all = catT + [sx3[:, 0, :, 10], sx3[:, 1, :, 10]]

    # p1 matmul + LN + relu -> zT
    pmv2 = posT.tile([128, gchunks, 2], F32, tag="pmv2")
    zpps = []
    for gc in range(gchunks):
        gsl = slice(gc * 128, (gc + 1) * 128)
        zpp = ps_pz.tile([128, 512], F32, tag="zpp")
        for kk in range(6):
            nc.tensor.matmul(
                zpp, _r(catT_all[kk][:, gsl]), _r(p1w[:, kk, :]),
                start=(kk == 0), stop=False,
            )
        nc.tensor.matmul(zpp, _r(ones1), _r(p1bR), start=False, stop=True)
        zsb = posT.tile([128, 512], F32, tag=f"zsb{gc}", name=f"zsb{gc}")
        nc.scalar.copy(out=zsb, in_=zpp)
        st6 = pos.tile([128, 6], F32, tag="pst6")
        nc.vector.bn_stats(out=st6, in_=zsb)
        nc.vector.bn_aggr(out=pmv2[:, gc, :], in_=st6)
        zpps.append(zsb)

    prr2 = posT.tile([128, gchunks], F32, tag="prr2")
    nc.vector.tensor_scalar(
        out=prr2, in0=pmv2[:, :, 1], scalar1=1.0, scalar2=1e-5,
        op0=OP.mult, op1=OP.add,
    )
    nc.scalar.activation(out=prr2, in_=prr2, func=AF.Sqrt)
    nc.vector.reciprocal(out=prr2, in_=prr2)

    for gc in range(gchunks):
        gsl = slice(gc * 128, (gc + 1) * 128)
        zpp = zpps[gc]
        tg = pos.tile([128, 512], F32, tag="ptg5")
        nc.vector.scalar_tensor_tensor(
            out=tg, in0=zpp, scalar=pmv2[:, gc, 0:1],
            in1=p1gB, op0=OP.subtract, op1=OP.mult,
        )
        nc.vector.scalar_tensor_tensor(
            out=tg, in0=tg, scalar=prr2[:, gc:gc + 1],
            in1=p1btB, op0=OP.mult, op1=OP.add,
        )
        rg = pos.tile([128, 512], F32, tag="prg5")
        nc.scalar.activation(out=rg, in_=tg, func=AF.Relu)
        for kk in range(4):
            ptp = ps_pt.tile([128, 128], F32, tag="pptp")
            nc.tensor.transpose(ptp, rg[:, kk * 128:(kk + 1) * 128], ident)
            nc.vector.tensor_copy(out=zT[kk][:, gsl], in_=ptp)

    # final projection
    for gc in range(gchunks):
        gsl = slice(gc * 128, (gc + 1) * 128)
        opp = ps_po.tile([128, 256], F32, tag="cpp", name="opp")
        for kk in range(4):
            nc.tensor.matmul(
                opp, _r(zT[kk][:, gsl]), _r(p2w[:, kk, :]),
                start=(kk == 0), stop=False,
            )
        nc.tensor.matmul(opp, _r(ones1), _r(p2bR), start=False, stop=True)
        osb = pos.tile([128, 256], F32, tag="osb")
        nc.vector.tensor_copy(out=osb, in_=opp)
        nc.sync.dma_start(out=out_d.ap()[gsl, :], in_=osb)


# ---------------------------------------------------------------------------
# host side
# ---------------------------------------------------------------------------

_NC_CACHE = {}
F8NP = ml_dtypes.float8_e4m3
BF16NP = ml_dtypes.bfloat16


def _get_nc(n_mega=FULL_N_MEGA, flags=frozenset()):
    key = (n_mega, flags)
    if key not in _NC_CACHE:
        _NC_CACHE[key] = build_nc(n_mega, flags)
    return _NC_CACHE[key]


def _flags(inp):
    f = set()
    if (np.any(inp["att_b1"]) or np.any(inp["piece_b1"])
            or np.any(inp["empty_b1"])):
        f.add("b1")
    if (np.any(inp["att_b2"]) or np.any(np.asarray(inp["piece_b2"]))
            or np.any(np.asarray(inp["empty_b2"]))):
        f.add("b2")
    if np.any(inp["sp_b"]):
        f.add("spb")
    if np.any(np.asarray(inp["sp_g"]) != 1.0) or np.any(inp["sp_beta"]):
        f.add("spg")
    return frozenset(f)


def _prep_weights(inp):
    f = np.float32
    att_W1 = np.asarray(inp["att_W1"], f)          # [8, 256, 64]
    att_b1 = np.asarray(inp["att_b1"], f)          # [8, 64]
    att_w2 = np.asarray(inp["att_w2"], f)          # [8, 64]
    att_b2 = np.asarray(inp["att_b2"], f)          # [8]
    piece_W1 = np.asarray(inp["piece_W1"], f)      # [256, 128]
    empty_W1 = np.asarray(inp["empty_W1"], f)
    piece_b1 = np.asarray(inp["piece_b1"], f)      # [128]
    empty_b1 = np.asarray(inp["empty_b1"], f)
    piece_w2 = np.asarray(inp["piece_w2"], f)      # [128]
    empty_w2 = np.asarray(inp["empty_w2"], f)
    c = np.ascontiguousarray
    S = np.float32(WSCALE)

    # attention/piece/empty hidden weights, fp8 DoubleRow packs, x16
    w1full = np.transpose(att_W1, (1, 0, 2)).reshape(256, 512)   # [C, hd]
    pef = np.concatenate([piece_W1, empty_W1], 1)                # [C, 256]
    w1dr = np.zeros((6, 128, 2, 128), F8NP)
    wf = (S * w1full).reshape(2, 128, 512).transpose(1, 0, 2)    # [k, i, hd]
    for m in range(4):
        w1dr[m] = wf[:, :, m * 128:(m + 1) * 128].astype(F8NP)
    pf = (S * pef).reshape(2, 128, 256).transpose(1, 0, 2)
    for m in range(2):
        w1dr[4 + m] = pf[:, :, m * 128:(m + 1) * 128].astype(F8NP)

    # scorer second layers, fp8 DoubleRow packs over hd chunks, x16
    w2dr = np.zeros((3, 128, 2, 32), np.float32)
    for h in range(H):
        ch = h // 2
        p, i = divmod(ch, 2)
        r0 = (h % 2) * 64
        w2dr[p, r0:r0 + 64, i, h] = S * att_w2[h]
    w2dr[2, :, 0, 8] = S * piece_w2
    w2dr[2, :, 1, 9] = S * empty_w2
    w2dr = w2dr.astype(F8NP)

    b1full = np.concatenate(
        [att_b1.reshape(512), piece_b1, empty_b1]
    )  # [768]
    b1c = c(S * b1full.reshape(6, 128).T)

    b2c = np.zeros((32, 1), f)
    b2c[0:8, 0] = 256.0 * att_b2
    b2c[8, 0] = 256.0 * np.float32(inp["piece_b2"])
    b2c[9, 0] = 256.0 * np.float32(inp["empty_b2"])

    spw8 = c(
        (S * np.asarray(inp["sp_W"], f)).reshape(2, 128, 256)
        .transpose(1, 0, 2).astype(F8NP)
    )

    return {
        "w1dr": c(w1dr), "w2dr": c(w2dr), "b1c": b1c, "b2c": b2c,
        "spw8": spw8,
        "spb16": c(S * np.asarray(inp["sp_b"], f).reshape(1, 256)),
        "spg": c(np.asarray(inp["sp_g"], f).reshape(1, 256)),
        "spbt": c(np.asarray(inp["sp_beta"], f).reshape(1, 256)),
        "sw": c((1.0 / (1.0 + np.exp(-np.asarray(inp["strat_w"], np.float64))))
                .astype(f).reshape(64, 1)),
        "cw": c(np.asarray(inp["comb_W"], f).reshape(16, 128, 256)),
        "cb": c(np.asarray(inp["comb_b"], f).reshape(1, 256)),
        "cg": c(np.asarray(inp["comb_g"], f).reshape(1, 256)),
        "cbt": c(np.asarray(inp["comb_beta"], f).reshape(1, 256)),
        "hw": c(np.asarray(inp["hier_W"], f).reshape(4, 128, 256)),
        "hb": c(np.asarray(inp["hier_b"], f).reshape(1, 256)),
        "hg": c(np.asarray(inp["hier_g"], f).reshape(1, 256)),
        "hbt": c(np.asarray(inp["hier_beta"], f).reshape(1, 256)),
        "p1w": c(np.asarray(inp["p1_W"], f).reshape(6, 128, 512)),
        "p1b": c(np.asarray(inp["p1_b"], f).reshape(1, 512)),
        "p1g": c(np.asarray(inp["p1_g"], f).reshape(1, 512)),
        "p1bt": c(np.asarray(inp["p1_beta"], f).reshape(1, 512)),
        "p2w": c(np.asarray(inp["p2_W"], f).reshape(4, 128, 256)),
        "p2b": c(np.asarray(inp["p2_b"], f).reshape(1, 256)),
    }


def make_in_maps(inputs, n_mega=FULL_N_MEGA):
    x = np.asarray(inputs["x"], np.float32)
    nt = np.asarray(inputs["node_types"]).astype(np.int32)
    wd = _prep_weights(inputs)
    nodes_pc = n_mega * MEGA * ST
    in_maps = []
    for cc in range(N_CORES):
        xc = x[cc * nodes_pc:(cc + 1) * nodes_pc]
        xT8 = np.ascontiguousarray(
            xc.T.reshape(2, 128, nodes_pc).transpose(1, 0, 2).astype(F8NP)
        )
        m = {
            "xT8": xT8,
            "xbf": np.ascontiguousarray(xc.astype(BF16NP)),
            "nt": np.ascontiguousarray(nt[cc * nodes_pc:(cc + 1) * nodes_pc]),
        }
        m.update(wd)
        in_maps.append(m)
    return in_maps


def run(inputs, n_mega=FULL_N_MEGA):
    nc = _get_nc(n_mega, _flags(inputs))
    in_maps = make_in_maps(inputs, n_mega)
    res = run_bass_kernel_spmd(nc, in_maps, core_ids=list(range(N_CORES)))
    return np.concatenate(
        [res.results[cc]["out"] for cc in range(N_CORES)], axis=0
    )


def kernel(**inputs):
    return run(inputs, FULL_N_MEGA)


# revision 16
# speedup vs baseline: 3.9223x; 3.9223x over previous
"""Trainium2 Bass kernel for nn_ChessGraphPooling (segment_reduce).

Data-parallel over whole graphs: 4096 boards x 64 nodes sharded across 8
NeuronCores (512 graphs / 32768 nodes per core); small weights replicated.

v2 design (vs. the streaming fp32r baseline):
  - x is shipped from the host in TWO layouts: node-layout bf16 [nodes, C]
    (pooling matmuls) and transposed fp8-e4m3 DoubleRow pairs
    [128, 2, nodes] (all per-node linears), eliminating all on-chip x
    transposes/copies and halving HBM traffic.
  - the three per-node matmuls (attention scorer hidden, piece/empty hidden,
    strategic projection) and the tiny scorer second layers run as fp8
    DoubleRow matmuls (2 fp8 contraction elements per PE cell).  Weights are
    scaled x16 on the host so they sit in fp8's normal range; the x256 score
    scale is removed inside the softmax exp, and the x16 strategic scale is
    absorbed exactly by the LayerNorm (eps adjusted by 256).
  - pooling uses x/sf as the matmul *stationary* operand so pooled features
    land directly in T-layout PSUM; they are staged to SBUF by DMA, not by
    engine copies.
  - scores are staged PSUM->SBUF by DMA as well; segment softmax is batched
    [80 rows, 8 graphs, 64 nodes] per megatile in bf16.
  - the strategic LN uses bn_stats then a single fused
    relu(zs*rho - mu*rho) activation per chunk on the scalar engine.
  - biases/LN affines that are all-zero/identity in this model are checked on
    the host; nonzero values compile a general (slower) variant.
"""

import os
import sys

sys.path.insert(0, "/opt/trn_rl_repo")

from contextlib import ExitStack

import numpy as np
import ml_dtypes

import concourse.bass as bass
import concourse.bacc as bacc
import concourse.tile as tile
import concourse.mybir as mybir
from concourse.bass_utils import run_bass_kernel_spmd
from concourse.masks import make_identity

F32 = mybir.dt.float32
F32R = mybir.dt.float32r
BF16 = mybir.dt.bfloat16
F8 = mybir.dt.float8e4
I32 = mybir.dt.int32
AF = mybir.ActivationFunctionType
OP = mybir.AluOpType
AX = mybir.AxisListType
DR = mybir.MatmulPerfMode.DoubleRow

C = 256
H = 8
NODES = 64
NEG = 0.2
N_CORES = 8
ST = 512          # nodes per supertile
CHUNKS = 4        # 128-node chunks per supertile
MEGA = 8          # supertiles per megatile (80 score rows)
FULL_N_MEGA = 8   # megatiles per core at full size
WSCALE = 16.0     # host-side fp8 weight scale

# scorer-hidden activations run on Act (DVE cannot fuse LReLU from PSUM);
# K_SF_DVE chunks of the strategic relu-affine run on DVE instead of Act
SF_DVE = int(os.environ.get("K_SF_DVE", "2"))


def _r(ap):
    return ap.bitcast(F32R)


def build_nc(n_mega=FULL_N_MEGA, flags=frozenset()):
    nodes_pc = n_mega * MEGA * ST
    graphs_pc = nodes_pc // NODES
    assert graphs_pc % 128 == 0, "post stage needs graphs_pc multiple of 128"

    nc = bacc.Bacc("TRN2", num_devices=N_CORES)

    dt = {}

    def din(name, shape, dtype=F32):
        dt[name] = nc.dram_tensor(name, shape, dtype, kind="ExternalInput")

    din("xT8", [128, 2, nodes_pc], F8)
    din("xbf", [nodes_pc, C], BF16)
    din("nt", [nodes_pc], I32)
    din("w1dr", [6, 128, 2, 128], F8)
    din("w2dr", [3, 128, 2, 32], F8)
    din("spw8", [128, 2, 256], F8)
    din("b1c", [128, 6])
    din("b2c", [32, 1])
    din("spb16", [1, 256], F32R)
    din("spg", [1, 256])
    din("spbt", [1, 256])
    din("sw", [64, 1])
    # post-stage weights (identical to baseline layout)
    din("cw", [16, 128, 256], F32R)
    din("cb", [1, 256], F32R)
    din("cg", [1, 256])
    din("cbt", [1, 256])
    din("hw", [4, 128, 256], F32R)
    din("hb", [1, 256], F32R)
    din("hg", [1, 256])
    din("hbt", [1, 256])
    din("p1w", [6, 128, 512], F32R)
    din("p1b", [1, 512], F32R)
    din("p1g", [1, 512])
    din("p1bt", [1, 512])
    din("p2w", [4, 128, 256], F32R)
    din("p2b", [1, 256], F32R)
    out_d = nc.dram_tensor("out", [graphs_pc, C], F32, kind="ExternalOutput")

    with tile.TileContext(nc) as tc:
        _build_body(nc, tc, n_mega, graphs_pc, dt, out_d, flags)
    nc.compile()
    return nc


def _bcast(nc, dst, src_d):
    nc.gpsimd.dma_start(
        out=dst, in_=src_d.ap().partition_broadcast(dst.shape[0])
    )


def _build_body(nc, tc, n_mega, graphs_pc, dt, out_d, flags):
    gchunks = graphs_pc // 128

    with ExitStack() as top:
        consts = top.enter_context(tc.tile_pool(name="consts", bufs=1))
        persist = top.enter_context(tc.tile_pool(name="persist", bufs=1))

        # ---- constants ----
        w1t = [consts.tile([128, 2, 128], F8, tag=f"w1t{m}", name=f"w1t{m}")
               for m in range(6)]
        for m in range(6):
            nc.sync.dma_start(out=w1t[m], in_=dt["w1dr"].ap()[m])
        w2t = [consts.tile([128, 2, 32], F8, tag=f"w2t{p}", name=f"w2t{p}")
               for p in range(3)]
        for p in range(3):
            nc.sync.dma_start(out=w2t[p], in_=dt["w2dr"].ap()[p])
        spwt = consts.tile([128, 2, 256], F8, tag="spwt")
        nc.sync.dma_start(out=spwt, in_=dt["spw8"].ap())
        b1c = consts.tile([128, 6], F32, tag="b1c")
        nc.sync.dma_start(out=b1c, in_=dt["b1c"].ap())
        b2c = consts.tile([32, 1], F32, tag="b2c")
        nc.sync.dma_start(out=b2c, in_=dt["b2c"].ap())
        spbr = consts.tile([1, 256], F32R, tag="spbr")
        nc.sync.dma_start(out=spbr, in_=dt["spb16"].ap())
        onesf = consts.tile([1, 128], F32, tag="onesf")
        nc.vector.memset(onesf, 1.0)
        ones1 = consts.tile([1, 128], F32R, tag="ones1")
        nc.vector.tensor_copy(out=ones1, in_=onesf)

        gB = consts.tile([128, 256], F32, tag="gB")
        _bcast(nc, gB, dt["spg"])
        btB = consts.tile([128, 256], F32, tag="btB")
        _bcast(nc, btB, dt["spbt"])

        sa = consts.tile([128, 1], F32, tag="sa")
        nc.sync.dma_start(out=sa[0:64, :], in_=dt["sw"].ap())
        nc.sync.dma_start(out=sa[64:128, :], in_=dt["sw"].ap())
        sa2 = consts.tile([128, 1], F32, tag="sa2")
        nc.vector.tensor_tensor(out=sa2, in0=sa, in1=sa, op=OP.mult)

        poolf = consts.tile([128, 2], F32, tag="poolf")
        nc.gpsimd.memset(poolf, 0.0)
        nc.gpsimd.memset(poolf[0:64, 0:1], 1.0 / NODES)
        nc.gpsimd.memset(poolf[64:128, 1:2], 1.0 / NODES)
        poolcol = consts.tile([128, 2], BF16, tag="poolcol")
        nc.vector.tensor_copy(out=poolcol, in_=poolf)

        identf = consts.tile([128, 128], F32, tag="identf")
        make_identity(nc, identf)
        identB = consts.tile([128, 128], BF16, tag="identB")
        nc.vector.tensor_copy(out=identB, in_=identf)

        maskS = consts.tile([128, 512], F32, tag="maskS")
        nc.vector.memset(maskS, 1.0)

        # ---- persistent staging (alive into the post stage) ----
        staged = persist.tile([128, 2, graphs_pc * 11], F32R, tag="staged")

        with ExitStack() as main:
            xtp = main.enter_context(tc.tile_pool(name="xtp", bufs=2))
            xbp = main.enter_context(tc.tile_pool(name="xbp", bufs=2))
            hlp = main.enter_context(tc.tile_pool(name="hlp", bufs=9))
            sfp = main.enter_context(tc.tile_pool(name="sfp", bufs=9))
            megap = main.enter_context(tc.tile_pool(name="megap", bufs=2))
            smp = main.enter_context(tc.tile_pool(name="smp", bufs=3))
            wcp = main.enter_context(tc.tile_pool(name="wcp", bufs=3))
            stp = main.enter_context(tc.tile_pool(name="stp", bufs=3))

            ps_ph = main.enter_context(
                tc.tile_pool(name="ps_ph", bufs=2, space="PSUM"))
            ps_sc = main.enter_context(
                tc.tile_pool(name="ps_sc", bufs=1, space="PSUM"))
            ps_pz = main.enter_context(
                tc.tile_pool(name="ps_pz", bufs=2, space="PSUM"))
            ps_pt = main.enter_context(
                tc.tile_pool(name="ps_pt", bufs=1, space="PSUM"))
            ps_tp = main.enter_context(
                tc.tile_pool(name="ps_tp", bufs=1, space="PSUM"))

            for mega in range(n_mega):
                _mega_body(
                    nc, tc, mega, dt, staged, flags,
                    w1t, w2t, spwt, b1c, b2c, spbr, gB, btB, sa, sa2,
                    poolcol, identB, maskS, ones1,
                    xtp, xbp, hlp, sfp, megap, smp, wcp, stp,
                    ps_ph, ps_sc, ps_pz, ps_pt, ps_tp,
                )

        # ---- post stage ----
        with ExitStack() as post:
            posw = post.enter_context(tc.tile_pool(name="posw", bufs=1))
            pos = post.enter_context(tc.tile_pool(name="pos", bufs=4))
            posT = post.enter_context(tc.tile_pool(name="posT", bufs=1))
            ps_po = post.enter_context(
                tc.tile_pool(name="ps_po", bufs=2, space="PSUM"))
            ps_pz2 = post.enter_context(
                tc.tile_pool(name="ps_pz2", bufs=2, space="PSUM"))
            ps_pt2 = post.enter_context(
                tc.tile_pool(name="ps_pt2", bufs=2, space="PSUM"))
            _post_body(
                nc, tc, graphs_pc, gchunks, dt, staged,
                ones1, identf, posw, pos, posT, ps_po, ps_pz2, ps_pt2, out_d,
            )


def _mega_body(
    nc, tc, mega, dt, staged, flags,
    w1t, w2t, spwt, b1c, b2c, spbr, gB, btB, sa, sa2,
    poolcol, identB, maskS, ones1,
    xtp, xbp, hlp, sfp, megap, smp, wcp, stp,
    ps_ph, ps_sc, ps_pz, ps_pt, ps_tp,
):
    mst = mega * MEGA          # first supertile of this mega
    mn0 = mst * ST             # first node

    # ---- input DMAs ----
    xT8m = xtp.tile([128, 2, MEGA * ST], F8, tag="xT8m")
    nc.sync.dma_start(
        out=xT8m, in_=dt["xT8"].ap()[:, :, mn0:mn0 + MEGA * ST]
    )
    xbfm = xbp.tile([128, MEGA * CHUNKS, 256], BF16, tag="xbfm")
    nc.sync.dma_start(
        out=xbfm,
        in_=dt["xbf"].ap()[mn0:mn0 + MEGA * ST, :]
        .rearrange("(s p) m -> p s m", p=128),
    )

    # masks: node_types -> maskS rows 10*j+8 (m) / 10*j+9 (1-m)
    ntm = megap.tile([8, 512], I32, tag="ntm")
    nc.sync.dma_start(
        out=ntm,
        in_=dt["nt"].ap()[mn0:mn0 + MEGA * ST].rearrange("(s n) -> s n", s=8),
    )
    m8 = megap.tile([8, 512], F32, tag="m8")
    nc.vector.tensor_copy(out=m8, in_=ntm)
    n8 = megap.tile([8, 512], F32, tag="n8")
    nc.vector.tensor_scalar(
        out=n8, in0=m8, scalar1=-1.0, scalar2=1.0, op0=OP.mult, op1=OP.add
    )
    nc.gpsimd.dma_start(out=maskS[8:80:10, :], in_=m8)
    nc.gpsimd.dma_start(out=maskS[9:80:10, :], in_=n8)

    # ---- phase 1: scorer hidden layers (fp8 DoubleRow), M-chunk outer ----
    hl8 = [hlp.tile([128, 3, 2, 512], F8, tag="hl8", name=f"hl8_{s}")
           for s in range(MEGA)]
    for m in range(6):
        pair, half = divmod(m, 2) if m < 4 else (2, m - 4)
        for s in range(MEGA):
            ph = ps_ph.tile([128, 512], F32, tag="ph")
            for h in range(2):
                nc.tensor.matmul(
                    ph[:, h * 256:(h + 1) * 256], w1t[m],
                    xT8m[:, :, s * ST + h * 256:s * ST + (h + 1) * 256],
                    start=True, stop=True, perf_mode=DR,
                )
            dst = hl8[s][:, pair, half, :]
            nc.scalar.activation(
                out=dst, in_=ph, func=AF.Prelu,
                bias=(b1c[:, m:m + 1] if "b1" in flags else 0.0),
                scale=1.0, alpha=NEG,
            )

    # ---- phase 2: scorer second layers -> scores (fp8 DoubleRow) ----
    scstack = megap.tile([80, 512], F32, tag="scstack")
    for s in range(MEGA):
        # separate PSUM tiles per node-half so the two accumulation groups
        # live in different banks
        scth = [ps_sc.tile([32, 256], F32, tag=f"sct{h}", name=f"sct{h}")
                for h in range(2)]
        for p in range(3):
            for h in range(2):
                nc.tensor.matmul(
                    scth[h], w2t[p],
                    hl8[s][:, p, :, h * 256:(h + 1) * 256],
                    start=(p == 0), stop=(p == 2), perf_mode=DR,
                )
        stmp = smp.tile([10, 512], F32, tag="stmp")
        for h in range(2):
            osl = stmp[:, h * 256:(h + 1) * 256]
            isl = scth[h][0:10, :]
            if "b2" in flags:
                nc.scalar.activation(
                    out=osl, in_=isl, func=AF.Identity,
                    bias=b2c[0:10, :], scale=1.0,
                )
            elif (s + h) % 2 == 0:
                nc.vector.tensor_copy(out=osl, in_=isl)
            else:
                nc.scalar.copy(out=osl, in_=isl)
        nc.sync.dma_start(out=scstack[s * 10:(s + 1) * 10, :], in_=stmp)

    # ---- phase 3: batched segment softmax (scores are 256x scaled) ----
    nc.gpsimd.tensor_tensor(
        out=scstack, in0=scstack, in1=maskS[0:80, :], op=OP.mult
    )
    sc3 = scstack.rearrange("p (g n) -> p g n", n=NODES)
    mx = megap.tile([80, 8], F32, tag="mx")
    nc.vector.tensor_reduce(out=mx, in_=sc3, axis=AX.X, op=OP.max)
    wsub = megap.tile([80, 512], F32, tag="wsub")
    nc.vector.tensor_tensor(
        out=wsub.rearrange("p (g n) -> p g n", n=NODES), in0=sc3,
        in1=mx.unsqueeze(2).broadcast_to([80, 8, NODES]),
        op=OP.subtract,
    )
    wT = megap.tile([80, 512], BF16, tag="wT")
    nc.scalar.activation(out=wT, in_=wsub, func=AF.Exp, scale=1.0 / 256.0)
    wT3 = wT.rearrange("p (g n) -> p g n", n=NODES)
    dsum = megap.tile([80, 8], F32, tag="dsum")
    nc.vector.tensor_reduce(out=dsum, in_=wT3, axis=AX.X, op=OP.add)
    nc.vector.tensor_scalar(
        out=dsum, in0=dsum, scalar1=1e-16, scalar2=None, op0=OP.add,
    )
    nc.vector.reciprocal(out=dsum, in_=dsum)
    dsb = megap.tile([80, 8], BF16, tag="dsb")
    nc.vector.tensor_copy(out=dsb, in_=dsum)
    nc.vector.tensor_tensor(
        out=wT3, in0=wT3,
        in1=dsb.unsqueeze(2).broadcast_to([80, 8, NODES]),
        op=OP.mult,
    )

    # transpose the weight stack: wtt[:, c, r] = wT[r, c*128+:]
    wtt = megap.tile([128, 4, 80], BF16, tag="wtt")
    for c in range(CHUNKS):
        tp = ps_tp.tile([128, 80], BF16, tag="tp")
        nc.tensor.transpose(
            tp, wT[:, c * 128:(c + 1) * 128], identB[0:80, 0:80]
        )
        nc.vector.tensor_copy(out=wtt[:, c, :], in_=tp)

    # ---- phase 4: strategic branch + pooling, per supertile ----
    mvs = megap.tile([128, MEGA, 4, 2], F32, tag="mvs")
    for s in range(MEGA):
        sg = mst + s
        # 4a: strat matmuls + LN stats
        pzs = []
        for cp in range(2):
            pz = ps_pz.tile([128, 2, 256], F32, tag="pz")
            pzs.append(pz)
            for cc in range(2):
                c = cp * 2 + cc
                psl = pz[:, cc, :]
                nc.tensor.matmul(
                    psl,
                    xT8m[:, :, s * ST + c * 128:s * ST + (c + 1) * 128],
                    spwt, start=True, stop=("spb" not in flags),
                    perf_mode=DR,
                )
                if "spb" in flags:
                    nc.tensor.matmul(
                        psl, _r(ones1), _r(spbr), start=False, stop=True,
                        skip_group_check=True,
                    )
                st6 = smp.tile([128, 6], F32, tag="st6")
                nc.vector.bn_stats(out=st6, in_=psl)
                nc.vector.bn_aggr(out=mvs[:, s, c, :], in_=st6)
        # rho chain for this supertile: rho = sa*rsqrt(sa^2*var' + 256*eps)
        rho = smp.tile([128, 4], F32, tag="rho")
        nc.vector.tensor_scalar(
            out=rho, in0=mvs[:, s, :, 1], scalar1=sa2, scalar2=256e-5,
            op0=OP.mult, op1=OP.add,
        )
        nc.scalar.activation(out=rho, in_=rho, func=AF.Sqrt)
        nc.vector.reciprocal(out=rho, in_=rho)
        nc.vector.tensor_scalar(
            out=rho, in0=rho, scalar1=sa, scalar2=None, op0=OP.mult
        )
        nmr = smp.tile([128, 4], F32, tag="nmr")
        nc.vector.tensor_tensor(out=nmr, in0=mvs[:, s, :, 0], in1=rho,
                                op=OP.mult)
        nc.vector.tensor_scalar(
            out=nmr, in0=nmr, scalar1=-1.0, scalar2=None, op0=OP.mult
        )
        # 4b: sf = relu((zs - mu) * rho) (+ general gamma/beta path)
        sf = sfp.tile([128, 4, 256], BF16, tag="sf", name=f"sf_{s}")
        for c in range(CHUNKS):
            psl = pzs[c // 2][:, c % 2, :]
            if "spg" not in flags:
                if c < SF_DVE:
                    # (zs - mu) in bf16, then fused *rho / relu at 4x
                    t0 = smp.tile([128, 256], BF16, tag="t0")
                    nc.vector.tensor_scalar(
                        out=t0, in0=psl, scalar1=mvs[:, s, c, 0:1],
                        scalar2=None, op0=OP.subtract,
                    )
                    nc.vector.tensor_scalar(
                        out=sf[:, c, :], in0=t0, scalar1=rho[:, c:c + 1],
                        scalar2=0.0, op0=OP.mult, op1=OP.max,
                    )
                else:
                    nc.scalar.activation(
                        out=sf[:, c, :], in_=psl, func=AF.Relu,
                        bias=nmr[:, c:c + 1], scale=rho[:, c:c + 1],
                    )
            else:
                t1 = smp.tile([128, 256], F32, tag="t1")
                nc.vector.tensor_scalar(
                    out=t1, in0=psl, scalar1=mvs[:, s, c, 0:1],
                    scalar2=None, op0=OP.subtract,
                )
                nc.vector.tensor_tensor(out=t1, in0=t1, in1=gB, op=OP.mult)
                nc.vector.scalar_tensor_tensor(
                    out=t1, in0=t1, scalar=rho[:, c:c + 1], in1=btB,
                    op0=OP.mult, op1=OP.add,
                )
                nc.scalar.activation(out=sf[:, c, :], in_=t1, func=AF.Relu)

        # pooling weights for this supertile (block-diag over graph pairs)
        wcols = wcp.tile([128, 4, 20], BF16, tag="wcols")
        nc.gpsimd.memset(wcols, 0.0)
        nc.gpsimd.tensor_copy(
            out=wcols[0:64, :, 0:10], in_=wtt[0:64, :, s * 10:(s + 1) * 10]
        )
        nc.gpsimd.tensor_copy(
            out=wcols[64:128, :, 10:20], in_=wtt[64:128, :, s * 10:(s + 1) * 10]
        )

        # pooled features in T-layout, graph-major: pT[:, h, (c*2+gg)*11 + j]
        # (j=0..9 pooled-x heads, j=10 sf mean)
        pT = ps_pt.tile([128, 2, 88], F32, tag="pT")
        pTg = pT.rearrange("p k (g t) -> p k g t", t=11)
        for c in range(CHUNKS):
            for h in range(2):
                nc.tensor.matmul(
                    pTg[:, h, c * 2:(c + 1) * 2, 0:10],
                    xbfm[:, s * 4 + c, h * 128:(h + 1) * 128],
                    wcols[:, c, :], start=True, stop=True,
                )
                nc.tensor.matmul(
                    pTg[:, h, c * 2:(c + 1) * 2, 10:11],
                    sf[:, c, h * 128:(h + 1) * 128],
                    poolcol, start=True, stop=True,
                )
        # stage to SBUF (graph g = sg*8 + c*2 + gg at columns g*11 + j)
        osl = staged[:, :, sg * 88:(sg + 1) * 88]
        if s % 2 == 0:
            nc.scalar.copy(out=osl, in_=pT)
        else:
            nc.vector.tensor_copy(out=osl, in_=pT)


def _post_body(
    nc, tc, graphs_pc, gchunks, dt, staged,
    ones1, ident, posw, pos, posT, ps_po, ps_pz, ps_pt, out_d,
):
    cw = posw.tile([128, 16, 256], F32R, tag="cw")
    nc.sync.dma_start(out=cw, in_=dt["cw"].ap().rearrange("k p c -> p k c"))
    hwt = posw.tile([128, 4, 256], F32R, tag="hwt")
    nc.sync.dma_start(out=hwt, in_=dt["hw"].ap().rearrange("k p c -> p k c"))
    p1w = posw.tile([128, 6, 512], F32R, tag="p1w")
    nc.sync.dma_start(out=p1w, in_=dt["p1w"].ap().rearrange("k p c -> p k c"))
    p2w = posw.tile([128, 4, 256], F32R, tag="p2w")
    nc.sync.dma_start(out=p2w, in_=dt["p2w"].ap().rearrange("k p c -> p k c"))
    cbR = posw.tile([1, 256], F32R, tag="cbR")
    nc.sync.dma_start(out=cbR, in_=dt["cb"].ap())
    hbR = posw.tile([1, 256], F32R, tag="hbR")
    nc.sync.dma_start(out=hbR, in_=dt["hb"].ap())
    p1bR = posw.tile([1, 512], F32R, tag="p1bR")
    nc.sync.dma_start(out=p1bR, in_=dt["p1b"].ap())
    p2bR = posw.tile([1, 256], F32R, tag="p2bR")
    nc.sync.dma_start(out=p2bR, in_=dt["p2b"].ap())
    cgB = posw.tile([128, 256], F32, tag="cgB")
    _bcast(nc, cgB, dt["cg"])
    cbtB = posw.tile([128, 256], F32, tag="cbtB")
    _bcast(nc, cbtB, dt["cbt"])
    hgB = posw.tile([128, 256], F32, tag="hgB")
    _bcast(nc, hgB, dt["hg"])
    hbtB = posw.tile([128, 256], F32, tag="hbtB")
    _bcast(nc, hbtB, dt["hbt"])
    p1gB = posw.tile([128, 512], F32, tag="p1gB")
    _bcast(nc, p1gB, dt["p1g"])
    p1btB = posw.tile([128, 512], F32, tag="p1btB")
    _bcast(nc, p1btB, dt["p1bt"])

    sx3 = staged.rearrange("p k (g t) -> p k g t", t=11)

    catT = [posT.tile([128, graphs_pc], F32R, tag=f"catT{i}", name=f"catT{i}")
            for i in range(4)]
    zT = [posT.tile([128, graphs_pc], F32R, tag=f"zT{i}", name=f"zT{i}")
          for i in range(4)]
    pmv = posT.tile([128, 2 * gchunks, 2], F32, tag="pmv")

    # comb + hier matmuls, LN stats
    cps = []
    for gc in range(gchunks):
        gsl = slice(gc * 128, (gc + 1) * 128)
        cpp = ps_po.tile([128, 256], F32, tag="cpp")
        for h in range(H):
            for k in range(2):
                nc.tensor.matmul(
                    cpp, _r(sx3[:, k, gsl, h]), _r(cw[:, h * 2 + k, :]),
                    start=(h == 0 and k == 0), stop=False,
                )
        nc.tensor.matmul(cpp, _r(ones1), _r(cbR), start=False, stop=True)
        hpp = ps_po.tile([128, 256], F32, tag="cpp")
        for k in range(2):
            nc.tensor.matmul(
                hpp, _r(sx3[:, k, gsl, 8]), _r(hwt[:, k, :]),
                start=(k == 0), stop=False,
            )
            nc.tensor.matmul(
                hpp, _r(sx3[:, k, gsl, 9]), _r(hwt[:, 2 + k, :]),
                start=False, stop=(k == 1),
            )
        nc.tensor.matmul(hpp, _r(ones1), _r(hbR), start=False, stop=True)
        csb = posT.tile([128, 256], F32, tag=f"csb{gc}", name=f"csb{gc}")
        nc.scalar.copy(out=csb, in_=cpp)
        hsb = posT.tile([128, 256], F32, tag=f"hsb{gc}", name=f"hsb{gc}")
        nc.scalar.copy(out=hsb, in_=hpp)
        for i, ppx in enumerate((csb, hsb)):
            st6 = pos.tile([128, 6], F32, tag="pst6")
            nc.vector.bn_stats(out=st6, in_=ppx)
            nc.vector.bn_aggr(out=pmv[:, gc * 2 + i, :], in_=st6)
        cps.append((csb, hsb))

    prr = posT.tile([128, 2 * gchunks], F32, tag="prr")
    nc.vector.tensor_scalar(
        out=prr, in0=pmv[:, :, 1], scalar1=1.0, scalar2=1e-5,
        op0=OP.mult, op1=OP.add,
    )
    nc.scalar.activation(out=prr, in_=prr, func=AF.Sqrt)
    nc.vector.reciprocal(out=prr, in_=prr)

    for gc in range(gchunks):
        gsl = slice(gc * 128, (gc + 1) * 128)
        cpp, hpp = cps[gc]
        for i, (ppx, ggB, bbB) in enumerate(
            ((cpp, cgB, cbtB), (hpp, hgB, hbtB))
        ):
            tg = pos.tile([128, 256], F32, tag="ptg")
            nc.vector.scalar_tensor_tensor(
                out=tg, in0=ppx, scalar=pmv[:, gc * 2 + i, 0:1],
                in1=ggB, op0=OP.subtract, op1=OP.mult,
            )
            nc.vector.scalar_tensor_tensor(
                out=tg, in0=tg, scalar=prr[:, gc * 2 + i:gc * 2 + i + 1],
                in1=bbB, op0=OP.mult, op1=OP.add,
            )
            rg = pos.tile([128, 256], F32, tag="prg")
            nc.scalar.activation(out=rg, in_=tg, func=AF.Relu)
            for cc in range(2):
                ptp = ps_pt.tile([128, 128], F32, tag="pptp")
                nc.tensor.transpose(ptp, rg[:, cc * 128:(cc + 1) * 128], ident)
                nc.vector.tensor_copy(out=catT[i * 2 + cc][:, gsl], in_=ptp)

    catT_all = catT + [sx3[:, 0, :, 10], sx3[:, 1, :, 10]]

    # p1 matmul + LN + relu -> zT
    pmv2 = posT.tile([128, gchunks, 2], F32, tag="pmv2")
    zpps = []
    for gc in range(gchunks):
        gsl = slice(gc * 128, (gc + 1) * 128)
        zpp = ps_pz.tile([128, 512], F32, tag="zpp")
        for kk in range(6):
            nc.tensor.matmul(
                zpp, _r(catT_all[kk][:, gsl]), _r(p1w[:, kk, :]),
                start=(kk == 0), stop=False,
            )
        nc.tensor.matmul(zpp, _r(ones1), _r(p1bR), start=False, stop=True)
        zsb = posT.tile([128, 512], F32, tag=f"zsb{gc}", name=f"zsb{gc}")
        nc.scalar.copy(out=zsb, in_=zpp)
        st6 = pos.tile([128, 6], F32, tag="pst6")
        nc.vector.bn_stats(out=st6, in_=zsb)
        nc.vector.bn_aggr(out=pmv2[:, gc, :], in_=st6)
        zpps.append(zsb)

    prr2 = posT.tile([128, gchunks], F32, tag="prr2")
    nc.vector.tensor_scalar(
        out=prr2, in0=pmv2[:, :, 1], scalar1=1.0, scalar2=1e-5,
        op0=OP.mult, op1=OP.add,
    )
    nc.scalar.activation(out=prr2, in_=prr2, func=AF.Sqrt)
    nc.vector.reciprocal(out=prr2, in_=prr2)

    for gc in range(gchunks):
        gsl = slice(gc * 128, (gc + 1) * 128)
        zpp = zpps[gc]
        tg = pos.tile([128, 512], F32, tag="ptg5")
        nc.vector.scalar_tensor_tensor(
            out=tg, in0=zpp, scalar=pmv2[:, gc, 0:1],
            in1=p1gB, op0=OP.subtract, op1=OP.mult,
        )
        nc.vector.scalar_tensor_tensor(
            out=tg, in0=tg, scalar=prr2[:, gc:gc + 1],
            in1=p1btB, op0=OP.mult, op1=OP.add,
        )
        rg = pos.tile([128, 512], F32, tag="prg5")
        nc.scalar.activation(out=rg, in_=tg, func=AF.Relu)
        for kk in range(4):
            ptp = ps_pt.tile([128, 128], F32, tag="pptp")
            nc.tensor.transpose(ptp, rg[:, kk * 128:(kk + 1) * 128], ident)
            nc.vector.tensor_copy(out=zT[kk][:, gsl], in_=ptp)

    # final projection
    for gc in range(gchunks):
        gsl = slice(gc * 128, (gc + 1) * 128)
        opp = ps_po.tile([128, 256], F32, tag="cpp", name="opp")
        for kk in range(4):
            nc.tensor.matmul(
                opp, _r(zT[kk][:, gsl]), _r(p2w[:, kk, :]),
                start=(kk == 0), stop=False,
            )
        nc.tensor.matmul(opp, _r(ones1), _r(p2bR), start=False, stop=True)
        osb = pos.tile([128, 256], F32, tag="osb")
        nc.vector.tensor_copy(out=osb, in_=opp)
        nc.sync.dma_start(out=out_d.ap()[gsl, :], in_=osb)


# ---------------------------------------------------------------------------
# host side
# ---------------------------------------------------------------------------

_NC_CACHE = {}
F8NP = ml_dtypes.float8_e4m3
BF16NP = ml_dtypes.bfloat16


def _get_nc(n_mega=FULL_N_MEGA, flags=frozenset()):
    key = (n_mega, flags)
    if key not in _NC_CACHE:
        _NC_CACHE[key] = build_nc(n_mega, flags)
    return _NC_CACHE[key]


def _flags(inp):
    f = set()
    if (np.any(inp["att_b1"]) or np.any(inp["piece_b1"])
            or np.any(inp["empty_b1"])):
        f.add("b1")
    if (np.any(inp["att_b2"]) or np.any(np.asarray(inp["piece_b2"]))
            or np.any(np.asarray(inp["empty_b2"]))):
        f.add("b2")
    if np.any(inp["sp_b"]):
        f.add("spb")
    if np.any(np.asarray(inp["sp_g"]) != 1.0) or np.any(inp["sp_beta"]):
        f.add("spg")
    return frozenset(f)


def _prep_weights(inp):
    f = np.float32
    att_W1 = np.asarray(inp["att_W1"], f)          # [8, 256, 64]
    att_b1 = np.asarray(inp["att_b1"], f)          # [8, 64]
    att_w2 = np.asarray(inp["att_w2"], f)          # [8, 64]
    att_b2 = np.asarray(inp["att_b2"], f)          # [8]
    piece_W1 = np.asarray(inp["piece_W1"], f)      # [256, 128]
    empty_W1 = np.asarray(inp["empty_W1"], f)
    piece_b1 = np.asarray(inp["piece_b1"], f)      # [128]
    empty_b1 = np.asarray(inp["empty_b1"], f)
    piece_w2 = np.asarray(inp["piece_w2"], f)      # [128]
    empty_w2 = np.asarray(inp["empty_w2"], f)
    c = np.ascontiguousarray
    S = np.float32(WSCALE)

    # attention/piece/empty hidden weights, fp8 DoubleRow packs, x16
    w1full = np.transpose(att_W1, (1, 0, 2)).reshape(256, 512)   # [C, hd]
    pef = np.concatenate([piece_W1, empty_W1], 1)                # [C, 256]
    w1dr = np.zeros((6, 128, 2, 128), F8NP)
    wf = (S * w1full).reshape(2, 128, 512).transpose(1, 0, 2)    # [k, i, hd]
    for m in range(4):
        w1dr[m] = wf[:, :, m * 128:(m + 1) * 128].astype(F8NP)
    pf = (S * pef).reshape(2, 128, 256).transpose(1, 0, 2)
    for m in range(2):
        w1dr[4 + m] = pf[:, :, m * 128:(m + 1) * 128].astype(F8NP)

    # scorer second layers, fp8 DoubleRow packs over hd chunks, x16
    w2dr = np.zeros((3, 128, 2, 32), np.float32)
    for h in range(H):
        ch = h // 2
        p, i = divmod(ch, 2)
        r0 = (h % 2) * 64
        w2dr[p, r0:r0 + 64, i, h] = S * att_w2[h]
    w2dr[2, :, 0, 8] = S * piece_w2
    w2dr[2, :, 1, 9] = S * empty_w2
    w2dr = w2dr.astype(F8NP)

    b1full = np.concatenate(
        [att_b1.reshape(512), piece_b1, empty_b1]
    )  # [768]
    b1c = c(S * b1full.reshape(6, 128).T)

    b2c = np.zeros((32, 1), f)
    b2c[0:8, 0] = 256.0 * att_b2
    b2c[8, 0] = 256.0 * np.float32(inp["piece_b2"])
    b2c[9, 0] = 256.0 * np.float32(inp["empty_b2"])

    spw8 = c(
        (S * np.asarray(inp["sp_W"], f)).reshape(2, 128, 256)
        .transpose(1, 0, 2).astype(F8NP)
    )

    return {
        "w1dr": c(w1dr), "w2dr": c(w2dr), "b1c": b1c, "b2c": b2c,
        "spw8": spw8,
        "spb16": c(S * np.asarray(inp["sp_b"], f).reshape(1, 256)),
        "spg": c(np.asarray(inp["sp_g"], f).reshape(1, 256)),
        "spbt": c(np.asarray(inp["sp_beta"], f).reshape(1, 256)),
        "sw": c((1.0 / (1.0 + np.exp(-np.asarray(inp["strat_w"], np.float64))))
                .astype(f).reshape(64, 1)),
        "cw": c(np.asarray(inp["comb_W"], f).reshape(16, 128, 256)),
        "cb": c(np.asarray(inp["comb_b"], f).reshape(1, 256)),
        "cg": c(np.asarray(inp["comb_g"], f).reshape(1, 256)),
        "cbt": c(np.asarray(inp["comb_beta"], f).reshape(1, 256)),
        "hw": c(np.asarray(inp["hier_W"], f).reshape(4, 128, 256)),
        "hb": c(np.asarray(inp["hier_b"], f).reshape(1, 256)),
        "hg": c(np.asarray(inp["hier_g"], f).reshape(1, 256)),
        "hbt": c(np.asarray(inp["hier_beta"], f).reshape(1, 256)),
        "p1w": c(np.asarray(inp["p1_W"], f).reshape(6, 128, 512)),
        "p1b": c(np.asarray(inp["p1_b"], f).reshape(1, 512)),
        "p1g": c(np.asarray(inp["p1_g"], f).reshape(1, 512)),
        "p1bt": c(np.asarray(inp["p1_beta"], f).reshape(1, 512)),
        "p2w": c(np.asarray(inp["p2_W"], f).reshape(4, 128, 256)),
        "p2b": c(np.asarray(inp["p2_b"], f).reshape(1, 256)),
    }


def make_in_maps(inputs, n_mega=FULL_N_MEGA):
    x = np.asarray(inputs["x"], np.float32)
    nt = np.asarray(inputs["node_types"]).astype(np.int32)
    wd = _prep_weights(inputs)
    nodes_pc = n_mega * MEGA * ST
    in_maps = []
    for cc in range(N_CORES):
        xc = x[cc * nodes_pc:(cc + 1) * nodes_pc]
        xT8 = np.ascontiguousarray(
            xc.T.reshape(2, 128, nodes_pc).transpose(1, 0, 2).astype(F8NP)
        )
        m = {
            "xT8": xT8,
            "xbf": np.ascontiguousarray(xc.astype(BF16NP)),
            "nt": np.ascontiguousarray(nt[cc * nodes_pc:(cc + 1) * nodes_pc]),
        }
        m.update(wd)
        in_maps.append(m)
    return in_maps


def run(inputs, n_mega=FULL_N_MEGA):
    nc = _get_nc(n_mega, _flags(inputs))
    in_maps = make_in_maps(inputs, n_mega)
    res = run_bass_kernel_spmd(nc, in_maps, core_ids=list(range(N_CORES)))
    return np.concatenate(
        [res.results[cc]["out"] for cc in range(N_CORES)], axis=0
    )


def kernel(**inputs):
    return run(inputs, FULL_N_MEGA)
